# revision 1
# baseline (speedup 1.0000x reference)
"""CorrCosine TRN2 kernel.

out[b, i, j, h, w] = <cur[b,:,i,j]/||cur[b,:,i,j]||, ref[b,:,h,w]/||ref[b,:,h,w]||>

Data-parallel over batch B=8 across the 8 NeuronCores; per core one
[4096 x 256] @ [256 x 4096] GEMM in fp32r (TF32) plus the two L2
normalizations, fused by pre-scaling both operands with 1/norm computed
on-chip (sum over C via an all-ones stationary matmul, which also leaves
the result broadcast across all 128 partitions).
"""

import numpy as np

from concourse import bacc, mybir
from concourse import tile
from concourse.bass_utils import run_bass_kernel_spmd

B, C, H, W = 8, 256, 64, 64
HW = H * W            # 4096
P = 128               # partitions
KT = C // P           # 2 k-tiles
FD = 512              # psum bank free dim (fp32)
NCH = HW // FD        # 8 column chunks
MT = HW // P          # 32 m-tiles
OBW = 4096            # output staging width (2 MiB DMAs)
IBW = 2048            # input DMA width (1 MiB chunks, lets norm start early)

f32 = mybir.dt.float32
f32r = mybir.dt.float32r
AF = mybir.ActivationFunctionType

_cached_nc = None


def _build():
    nc = bacc.Bacc("TRN2", target_bir_lowering=False, debug=False)
    cur_d = nc.dram_tensor("cur", [C, HW], f32, kind="ExternalInput")
    ref_d = nc.dram_tensor("ref", [C, HW], f32, kind="ExternalInput")
    out_d = nc.dram_tensor("out", [HW, HW], f32, kind="ExternalOutput")

    with tile.TileContext(nc) as tc:
        with (
            tc.tile_pool(name="scl", bufs=1) as sclp,
            tc.tile_pool(name="cst", bufs=1) as cstp,
            tc.tile_pool(name="ps", bufs=8, space="PSUM") as psp,
        ):
            ones_f = cstp.tile([P, P], f32, tag="ones_f", name="ones_f")
            nc.gpsimd.memset(ones_f[:], 1.0)
            ones = cstp.tile([P, P], f32r, tag="ones", name="ones")
            nc.vector.tensor_copy(ones[:], ones_f[:])

            # ref gets pre-scaled (column scaling); cur is consumed raw (f32r)
            # and its 1/norm is applied as a per-partition scale during PSUM
            # evacuation instead.
            scl = {}
            for k in range(KT):
                scl["ref", k] = sclp.tile([P, HW], f32r, tag=f"sref{k}", name=f"scl_ref{k}")
            cur_r = {}
            for k in range(KT):
                cur_r[k] = sclp.tile([P, HW], f32r, tag=f"curr{k}", name=f"cur_r{k}")
            # inv_cur in column layout: invcur_col[p, m] = 1/||cur[:, m*128+p]||
            invcur = cstp.tile([P, MT], f32, tag="invcur", name="invcur")

            # --- normalization: per 512-column chunk, both k-tiles ---
            # ref first so the main GEMM (which needs every ref chunk but
            # only one cur chunk per m-tile) can start as early as possible.
            with (
                tc.tile_pool(name="raw", bufs=1) as rawp,
                tc.tile_pool(name="sq", bufs=3) as sqp,
                tc.tile_pool(name="nrm", bufs=2) as nrmp,
            ):
                raw = {}
                for k in range(KT):
                    raw["ref", k] = rawp.tile(
                        [P, HW], f32, tag=f"ref{k}", name=f"raw_ref{k}"
                    )
                # halves-first order: ref h0 x2 -> cur h0 x2 -> ref h1 -> cur h1,
                # so ref-chunk normalization starts after just two 1 MiB DMAs.
                # cur is DMA-cast straight to f32r (SWDGE dtype cast).
                for i in range(HW // IBW):
                    for k in range(KT):
                        nc.gpsimd.dma_start(
                            raw["ref", k][:, i * IBW:(i + 1) * IBW],
                            ref_d[k * P:(k + 1) * P, i * IBW:(i + 1) * IBW],
                        )
                    for k in range(KT):
                        nc.gpsimd.dma_start(
                            cur_r[k][:, i * IBW:(i + 1) * IBW],
                            cur_d[k * P:(k + 1) * P, i * IBW:(i + 1) * IBW],
                        )

                def norm_ref_chunk(ch):
                    sl = slice(ch * FD, (ch + 1) * FD)
                    sq0 = sqp.tile([P, FD], f32r, tag="sq", name="sq0")
                    nc.scalar.activation(sq0[:], raw["ref", 0][:, sl], AF.Square)
                    sq1 = sqp.tile([P, FD], f32r, tag="sq", name="sq1")
                    nc.scalar.activation(sq1[:], raw["ref", 1][:, sl], AF.Square)
                    # sum over C: ones.T @ sq, broadcast on all partitions
                    ss = psp.tile([P, FD], f32, tag="ss", name="ss", bufs=2)
                    nc.tensor.matmul(ss[:], ones[:], sq0[:], start=True, stop=False)
                    nc.tensor.matmul(ss[:], ones[:], sq1[:], start=False, stop=True)
                    nrm = nrmp.tile([P, FD], f32, tag="nrm", name="nrm")
                    nc.scalar.activation(nrm[:], ss[:], AF.Sqrt)
                    inv = nrmp.tile([P, FD], f32, tag="inv", name="inv")
                    nc.vector.reciprocal_approx_fast(inv[:], nrm[:])
                    # scale-muls on the otherwise-idle GpSimd engine, keeping
                    # DVE/ACT free for the GEMM's PSUM evacuation copies
                    nc.gpsimd.tensor_mul(scl["ref", 0][:, sl], raw["ref", 0][:, sl], inv[:])
                    nc.gpsimd.tensor_mul(scl["ref", 1][:, sl], raw["ref", 1][:, sl], inv[:])

                def norm_cur_chunk(ch):
                    # squares of the cur chunk (plain f32), then per-m-tile
                    # column sums via fp32 matmul: sq stationary, ones vector
                    # moving -> psum [128, 4] column layout; sqrt + 1/x.
                    sl = slice(ch * FD, (ch + 1) * FD)
                    sq0 = sqp.tile([P, FD], f32, tag="sq", name="sq0")
                    nc.scalar.activation(sq0[:], cur_r[0][:, sl], AF.Square)
                    sq1 = sqp.tile([P, FD], f32, tag="sq", name="sq1")
                    nc.scalar.activation(sq1[:], cur_r[1][:, sl], AF.Square)
                    mpc = FD // P  # m-tiles per chunk (4)
                    pc = psp.tile([P, mpc], f32, tag="ss", name="pc", bufs=2)
                    for q in range(mpc):
                        qsl = slice(q * P, (q + 1) * P)
                        nc.tensor.matmul(
                            pc[:, q:q + 1], sq0[:, qsl], ones_f[:, 0:1],
                            start=True, stop=False,
                        )
                        nc.tensor.matmul(
                            pc[:, q:q + 1], sq1[:, qsl], ones_f[:, 0:1],
                            start=False, stop=True,
                        )
                    ncol = nrmp.tile([P, mpc], f32, tag="ncol", name="ncol")
                    nc.scalar.activation(ncol[:], pc[:], AF.Sqrt)
                    nc.vector.reciprocal_approx_fast(
                        invcur[:, ch * mpc:(ch + 1) * mpc], ncol[:]
                    )

                for ch in range(NCH):
                    norm_ref_chunk(ch)

                # --- main GEMM: out[m*128 :, :] = inv_cur[m] * cur.T @ ref_s ---
                # interleaved with cur normalization: chunk ch of cur feeds
                # m-tiles 4ch..4ch+3, so out-DMA starts after ~9/16 of norm.
                with tc.tile_pool(name="outp", bufs=3) as outp:
                    ndma = 0
                    for m in range(MT):
                        if m % (MT // NCH) == 0:
                            norm_cur_chunk(m // (MT // NCH))
                        msl = slice(m * P, (m + 1) * P)
                        mscale = invcur[:, m:m + 1]
                        for half in range(HW // OBW):
                            ob = outp.tile([P, OBW], f32, tag="ob", name="ob")
                            # 2-bank psum tiles: 4 matmuls in, one wide copy out
                            for q in range(OBW // (2 * FD)):
                                pt = psp.tile(
                                    [P, 2 * FD], f32, tag="pt", name="pt", bufs=3
                                )
                                for sub in range(2):
                                    n = half * (OBW // FD) + q * 2 + sub
                                    nsl = slice(n * FD, (n + 1) * FD)
                                    psl = slice(sub * FD, (sub + 1) * FD)
                                    nc.tensor.matmul(
                                        pt[:, psl], cur_r[0][:, msl],
                                        scl["ref", 0][:, nsl],
                                        start=True, stop=False,
                                    )
                                    nc.tensor.matmul(
                                        pt[:, psl], cur_r[1][:, msl],
                                        scl["ref", 1][:, nsl],
                                        start=False, stop=True,
                                    )
                                osl = slice(q * 2 * FD, (q + 1) * 2 * FD)
                                # evacuate with the cur row scale fused in,
                                # balanced between ACT and DVE
                                if q % 2 == 0:
                                    nc.scalar.activation(
                                        ob[:, osl], pt[:], AF.Copy, scale=mscale
                                    )
                                else:
                                    nc.vector.tensor_scalar_mul(
                                        ob[:, osl], pt[:], mscale
                                    )
                            # alternate the two HWDGE rings (SP / ACT)
                            eng = nc.sync if ndma % 2 == 0 else nc.scalar
                            ndma += 1
                            eng.dma_start(
                                out_d[msl, half * OBW:(half + 1) * OBW], ob[:]
                            )

    nc.compile()
    return nc


def _get_nc():
    global _cached_nc
    if _cached_nc is None:
        _cached_nc = _build()
    return _cached_nc


def _run(cur, ref, trace=False, **kw):
    """cur/ref: [B, C, HW] float32. Returns (out [B, HW, HW], results)."""
    nc = _get_nc()
    in_maps = [{"cur": cur[b], "ref": ref[b]} for b in range(B)]
    res = run_bass_kernel_spmd(nc, in_maps, list(range(B)), trace=trace, **kw)
    out = np.stack([res.results[b]["out"] for b in range(B)])
    return out, res


def kernel(ref_features, cur_features):
    ref = np.ascontiguousarray(np.asarray(ref_features, np.float32).reshape(B, C, HW))
    cur = np.ascontiguousarray(np.asarray(cur_features, np.float32).reshape(B, C, HW))
    out, _ = _run(cur, ref)
    return out.reshape(B, H, W, H, W)



# revision 29
# speedup vs baseline: 1.4970x; 1.4970x over previous
"""CorrCosine TRN2 kernel.

out[b, i, j, h, w] = <cur[b,:,i,j]/||cur[b,:,i,j]||, ref[b,:,h,w]/||ref[b,:,h,w]||>

Data-parallel over batch B=8 across the 8 NeuronCores; per core one
[4096 x 256] @ [256 x 4096] GEMM plus the two L2 normalizations.

All I/O and the GEMM run in fp16 (tolerance is 2e-2; fp16 keeps the
rel err ~1e-3): inputs are host-cast to fp16 (4 MiB/core), the output
is written as fp16 (32 MiB/core) and host-cast back to fp32. That
halves the HBM traffic of the fp32 version, which was DMA-bound; the
kernel is then tensor-bound at ~1 col/cycle.

Both operands are pre-scaled by their 1/norm (computed on-chip via an
all-ones stationary matmul, which leaves the column sums broadcast
across all 128 partitions), so PSUM evacuation is a pure fp32->fp16
copy split between ACT and DVE.
"""

import numpy as np

from concourse import bacc, mybir
from concourse import tile
from concourse.bass_utils import run_bass_kernel_spmd

B, C, H, W = 8, 256, 64, 64
HW = H * W            # 4096
P = 128               # partitions
KT = C // P           # 2 k-tiles
FD = 512              # psum bank free dim (fp32)
NCH = HW // FD        # 8 column chunks
MT = HW // P          # 32 m-tiles
OBW = 4096            # output staging width
IBW = 1024            # input DMA width (256 KiB chunks, lets norm start early)

f32 = mybir.dt.float32
f16 = mybir.dt.float16
AF = mybir.ActivationFunctionType

_cached_nc = None


def _build():
    nc = bacc.Bacc("TRN2", target_bir_lowering=False, debug=False)
    cur_d = nc.dram_tensor("cur", [C, HW], f16, kind="ExternalInput")
    ref_d = nc.dram_tensor("ref", [C, HW], f16, kind="ExternalInput")
    out_d = nc.dram_tensor("out", [HW, HW], f16, kind="ExternalOutput")

    with tile.TileContext(nc) as tc:
        with (
            tc.tile_pool(name="scl", bufs=1) as sclp,
            tc.tile_pool(name="cst", bufs=1) as cstp,
            tc.tile_pool(name="ps", bufs=8, space="PSUM") as psp,
        ):
            ones_f = cstp.tile([P, FD], f32, tag="ones_f", name="ones_f")
            nc.gpsimd.memset(ones_f[:], 1.0)
            ones_w = cstp.tile([P, FD], f16, tag="ones_w", name="ones_w")
            nc.vector.tensor_copy(ones_w[:], ones_f[:])
            ones = ones_w[:, 0:P]

            # raw + 1/norm-scaled copies of both operands, fp16
            raw = {}
            scl = {}
            for t in ("ref", "cur"):
                for k in range(KT):
                    raw[t, k] = sclp.tile([P, HW], f16, tag=f"raw_{t}{k}", name=f"raw_{t}{k}")
                    scl[t, k] = sclp.tile([P, HW], f16, tag=f"scl_{t}{k}", name=f"scl_{t}{k}")

            with (
                tc.tile_pool(name="sq", bufs=3) as sqp,
                tc.tile_pool(name="nrm", bufs=2) as nrmp,
            ):
                # cur q0 first (feeds cur chunk 0+1 norms), then all of ref
                # (the GEMM streams ALL of ref per m-tile), then the rest of
                # cur; HWDGE ring (SP): ~free enqueue and fast descriptor
                # processing vs gpsimd's SWDGE.
                dram = {"ref": ref_d, "cur": cur_d}
                NQ = HW // IBW
                order = [("cur", 0)] + [("ref", i) for i in range(NQ)] \
                    + [("cur", i) for i in range(1, NQ)]
                for t, i in order:
                    for k in range(KT):
                        nc.sync.dma_start(
                            raw[t, k][:, i * IBW:(i + 1) * IBW],
                            dram[t][k * P:(k + 1) * P, i * IBW:(i + 1) * IBW],
                        )

                def norm_chunk(t, ch, steady=False):
                    # per 512-col chunk of tensor t: square both k-tiles,
                    # sum over C via ones.T @ sq (the result is broadcast on
                    # all 128 partitions), one fused Rsqrt on ACT (fp16 out
                    # so the scale-muls run at pure-fp16 DVE speed), scale.
                    # steady=True (cur chunks during the GEMM) pushes the
                    # elementwise work to the otherwise-idle gpsimd so ACT/
                    # DVE stay free for PSUM evacuation.
                    sl = slice(ch * FD, (ch + 1) * FD)
                    sq0 = sqp.tile([P, FD], f16, tag="sq", name="sq0")
                    nc.scalar.activation(sq0[:], raw[t, 0][:, sl], AF.Square)
                    sq1 = sqp.tile([P, FD], f16, tag="sq", name="sq1")
                    e_sq1 = nc.gpsimd if steady else nc.vector
                    e_sq1.tensor_mul(sq1[:], raw[t, 1][:, sl], raw[t, 1][:, sl])
                    # ss shares the GEMM's psum rotation (tag "pt") so the
                    # GEMM gets all 8 banks
                    ss = psp.tile([P, 2 * FD], f32, tag="pt", name="ss", bufs=4)
                    nc.tensor.matmul(ss[:, 0:FD], ones, sq0[:], start=True, stop=False)
                    nc.tensor.matmul(ss[:, 0:FD], ones, sq1[:], start=False, stop=True)
                    nrm = nrmp.tile([P, FD], f32, tag="nrm", name="nrm")
                    nc.scalar.activation(nrm[:], ss[:, 0:FD], AF.Sqrt)
                    inv = nrmp.tile([P, FD], f32, tag="inv", name="inv")
                    nc.vector.reciprocal_approx_fast(inv[:], nrm[:])
                    if steady:
                        nc.gpsimd.tensor_mul(scl[t, 0][:, sl], raw[t, 0][:, sl], inv[:])
                        nc.gpsimd.tensor_mul(scl[t, 1][:, sl], raw[t, 1][:, sl], inv[:])
                    elif t == "ref":
                        nc.gpsimd.tensor_mul(scl[t, 0][:, sl], raw[t, 0][:, sl], inv[:])
                        nc.vector.tensor_mul(scl[t, 1][:, sl], raw[t, 1][:, sl], inv[:])
                    else:
                        nc.vector.tensor_mul(scl[t, 0][:, sl], raw[t, 0][:, sl], inv[:])
                        nc.vector.tensor_mul(scl[t, 1][:, sl], raw[t, 1][:, sl], inv[:])

                # PE warmup: dummy matmuls into the ss rotation while the
                # first input DMAs are in flight, so the HAM clock-gate is
                # released (1.2 -> 2.4 GHz) before the real matmuls start.
                for w in range(8):
                    wss = psp.tile([P, 2 * FD], f32, tag="pt", name="wss", bufs=4)
                    nc.tensor.matmul(wss[:, 0:FD], ones, ones_w[:], start=True, stop=True)

                for ch in range(NCH):
                    norm_chunk("ref", ch)
                norm_chunk("cur", 0)

                # --- main GEMM: out[m*128:, :] = scl_cur[:, m].T @ scl_ref ---
                # ref norm chunk n is issued just before the m=0 matmuls that
                # consume it, so m-tile 0 RIDES the norm pipeline (the PE
                # FIFO would otherwise stall behind all 8 ref chunks).
                # cur normalization runs one chunk AHEAD of the GEMM group
                # that consumes it (chunk ch feeds m-tiles 4ch..4ch+3 as
                # stationary), so its chain hides under 4 m-tiles of matmuls
                # instead of stalling PE at each chunk boundary.
                with tc.tile_pool(name="outp", bufs=3) as outp:
                    ndma = 0
                    for m in range(MT):
                        if m % (MT // NCH) == 0:
                            ch = m // (MT // NCH)
                            if ch + 1 < NCH:
                                norm_chunk("cur", ch + 1, steady=True)
                        msl = slice(m * P, (m + 1) * P)
                        ob = outp.tile([P, OBW], f16, tag="ob", name="ob")
                        # 2-bank psum tiles: 4 matmuls in, one wide copy out
                        for q in range(OBW // (2 * FD)):
                            pt = psp.tile(
                                [P, 2 * FD], f32, tag="pt", name="pt", bufs=4
                            )
                            for sub in range(2):
                                n = q * 2 + sub
                                nsl = slice(n * FD, (n + 1) * FD)
                                psl = slice(sub * FD, (sub + 1) * FD)
                                nc.tensor.matmul(
                                    pt[:, psl], scl["cur", 0][:, msl],
                                    scl["ref", 0][:, nsl],
                                    start=True, stop=False,
                                )
                                nc.tensor.matmul(
                                    pt[:, psl], scl["cur", 1][:, msl],
                                    scl["ref", 1][:, nsl],
                                    start=False, stop=True,
                                )
                            osl = slice(q * 2 * FD, (q + 1) * 2 * FD)
                            # evacuate (pure fp32->fp16 copy), balanced
                            # between ACT and DVE
                            if q % 2 == 0:
                                nc.scalar.activation(ob[:, osl], pt[:], AF.Copy)
                            else:
                                nc.vector.tensor_copy(ob[:, osl], pt[:])
                            # quarter-width output DMAs: earlier start,
                            # shorter tail; alternate the HWDGE rings
                            eng = nc.sync if ndma % 2 == 0 else nc.scalar
                            ndma += 1
                            eng.dma_start(out_d[msl, osl], ob[:, osl])

    nc.compile()
    return nc


def _get_nc():
    global _cached_nc
    if _cached_nc is None:
        _cached_nc = _build()
    return _cached_nc


def _run(cur, ref, trace=False, **kw):
    """cur/ref: [B, C, HW] float16. Returns (out [B, HW, HW] f16, results)."""
    nc = _get_nc()
    in_maps = [{"cur": cur[b], "ref": ref[b]} for b in range(B)]
    res = run_bass_kernel_spmd(nc, in_maps, list(range(B)), trace=trace, **kw)
    out = np.stack([np.asarray(res.results[b]["out"]) for b in range(B)])
    return out, res


def kernel(ref_features, cur_features):
    ref = np.ascontiguousarray(
        np.asarray(ref_features, np.float32).reshape(B, C, HW).astype(np.float16)
    )
    cur = np.ascontiguousarray(
        np.asarray(cur_features, np.float32).reshape(B, C, HW).astype(np.float16)
    )
    out, _ = _run(cur, ref)
    return out.astype(np.float32).reshape(B, H, W, H, W)


# revision 30
# speedup vs baseline: 1.5060x; 1.0061x over previous
"""CorrCosine TRN2 kernel.

out[b, i, j, h, w] = <cur[b,:,i,j]/||cur[b,:,i,j]||, ref[b,:,h,w]/||ref[b,:,h,w]||>

Data-parallel over batch B=8 across the 8 NeuronCores; per core one
[4096 x 256] @ [256 x 4096] GEMM plus the two L2 normalizations.

All I/O and the GEMM run in fp16 (tolerance is 2e-2; fp16 keeps the
rel err ~1e-3): inputs are host-cast to fp16 (4 MiB/core), the output
is written as fp16 (32 MiB/core) and host-cast back to fp32. That
halves the HBM traffic of the fp32 version, which was DMA-bound; the
kernel is then tensor-bound at ~1 col/cycle.

Both operands are pre-scaled by their 1/norm (computed on-chip via an
all-ones stationary matmul, which leaves the column sums broadcast
across all 128 partitions), so PSUM evacuation is a pure fp32->fp16
copy split between ACT and DVE.
"""

import numpy as np

from concourse import bacc, mybir
from concourse import tile
from concourse.bass_utils import run_bass_kernel_spmd

B, C, H, W = 8, 256, 64, 64
HW = H * W            # 4096
P = 128               # partitions
KT = C // P           # 2 k-tiles
FD = 512              # psum bank free dim (fp32)
NCH = HW // FD        # 8 column chunks
MT = HW // P          # 32 m-tiles
OBW = 4096            # output staging width
IBW = 1024            # input DMA width (256 KiB chunks, lets norm start early)

f32 = mybir.dt.float32
f16 = mybir.dt.float16
AF = mybir.ActivationFunctionType

_cached_nc = None


def _build():
    nc = bacc.Bacc("TRN2", target_bir_lowering=False, debug=False)
    cur_d = nc.dram_tensor("cur", [C, HW], f16, kind="ExternalInput")
    ref_d = nc.dram_tensor("ref", [C, HW], f16, kind="ExternalInput")
    out_d = nc.dram_tensor("out", [HW, HW], f16, kind="ExternalOutput")

    with tile.TileContext(nc) as tc:
        with (
            tc.tile_pool(name="scl", bufs=1) as sclp,
            tc.tile_pool(name="cst", bufs=1) as cstp,
            tc.tile_pool(name="ps", bufs=8, space="PSUM") as psp,
        ):
            ones_f = cstp.tile([P, FD], f32, tag="ones_f", name="ones_f")
            nc.gpsimd.memset(ones_f[:], 1.0)
            ones_w = cstp.tile([P, FD], f16, tag="ones_w", name="ones_w")
            nc.vector.tensor_copy(ones_w[:], ones_f[:])
            ones = ones_w[:, 0:P]

            # raw + 1/norm-scaled copies of both operands, fp16
            raw = {}
            scl = {}
            for t in ("ref", "cur"):
                for k in range(KT):
                    raw[t, k] = sclp.tile([P, HW], f16, tag=f"raw_{t}{k}", name=f"raw_{t}{k}")
                    scl[t, k] = sclp.tile([P, HW], f16, tag=f"scl_{t}{k}", name=f"scl_{t}{k}")

            with (
                tc.tile_pool(name="sq", bufs=3) as sqp,
                tc.tile_pool(name="nrm", bufs=2) as nrmp,
            ):
                # cur q0 first (feeds cur chunk 0+1 norms), then all of ref
                # (the GEMM streams ALL of ref per m-tile), then the rest of
                # cur; HWDGE ring (SP): ~free enqueue and fast descriptor
                # processing vs gpsimd's SWDGE.
                dram = {"ref": ref_d, "cur": cur_d}
                NQ = HW // IBW
                order = [("cur", 0)] + [("ref", i) for i in range(NQ)] \
                    + [("cur", i) for i in range(1, NQ)]
                for t, i in order:
                    for k in range(KT):
                        nc.sync.dma_start(
                            raw[t, k][:, i * IBW:(i + 1) * IBW],
                            dram[t][k * P:(k + 1) * P, i * IBW:(i + 1) * IBW],
                        )

                def norm_chunk(t, ch, steady=False):
                    # per 512-col chunk of tensor t: square both k-tiles,
                    # sum over C via ones.T @ sq (the result is broadcast on
                    # all 128 partitions), one fused Rsqrt on ACT (fp16 out
                    # so the scale-muls run at pure-fp16 DVE speed), scale.
                    # steady=True (cur chunks during the GEMM) pushes the
                    # elementwise work to the otherwise-idle gpsimd so ACT/
                    # DVE stay free for PSUM evacuation.
                    sl = slice(ch * FD, (ch + 1) * FD)
                    sq0 = sqp.tile([P, FD], f16, tag="sq", name="sq0")
                    nc.scalar.activation(sq0[:], raw[t, 0][:, sl], AF.Square)
                    sq1 = sqp.tile([P, FD], f16, tag="sq", name="sq1")
                    e_sq1 = nc.gpsimd if steady else nc.vector
                    e_sq1.tensor_mul(sq1[:], raw[t, 1][:, sl], raw[t, 1][:, sl])
                    # ss shares the GEMM's psum rotation (tag "pt") so the
                    # GEMM gets all 8 banks
                    ss = psp.tile([P, 2 * FD], f32, tag="pt", name="ss", bufs=4)
                    nc.tensor.matmul(ss[:, 0:FD], ones, sq0[:], start=True, stop=False)
                    nc.tensor.matmul(ss[:, 0:FD], ones, sq1[:], start=False, stop=True)
                    nrm = nrmp.tile([P, FD], f32, tag="nrm", name="nrm")
                    nc.scalar.activation(nrm[:], ss[:, 0:FD], AF.Sqrt)
                    inv = nrmp.tile([P, FD], f32, tag="inv", name="inv")
                    nc.vector.reciprocal_approx_fast(inv[:], nrm[:])
                    if steady:
                        nc.gpsimd.tensor_mul(scl[t, 0][:, sl], raw[t, 0][:, sl], inv[:])
                        nc.gpsimd.tensor_mul(scl[t, 1][:, sl], raw[t, 1][:, sl], inv[:])
                    elif t == "ref":
                        # balance the 16 scale-muls: gpsimd takes 1.5 per
                        # chunk on average, DVE (busy with sq1+recip) 0.5
                        nc.gpsimd.tensor_mul(scl[t, 0][:, sl], raw[t, 0][:, sl], inv[:])
                        e_m1 = nc.vector if ch % 2 == 0 else nc.gpsimd
                        e_m1.tensor_mul(scl[t, 1][:, sl], raw[t, 1][:, sl], inv[:])
                    else:
                        nc.vector.tensor_mul(scl[t, 0][:, sl], raw[t, 0][:, sl], inv[:])
                        nc.vector.tensor_mul(scl[t, 1][:, sl], raw[t, 1][:, sl], inv[:])

                # PE warmup: dummy matmuls into the ss rotation while the
                # first input DMAs are in flight, so the HAM clock-gate is
                # released (1.2 -> 2.4 GHz) before the real matmuls start.
                for w in range(8):
                    wss = psp.tile([P, 2 * FD], f32, tag="pt", name="wss", bufs=4)
                    nc.tensor.matmul(wss[:, 0:FD], ones, ones_w[:], start=True, stop=True)

                for ch in range(NCH):
                    norm_chunk("ref", ch)
                norm_chunk("cur", 0)

                # --- main GEMM: out[m*128:, :] = scl_cur[:, m].T @ scl_ref ---
                # ref norm chunk n is issued just before the m=0 matmuls that
                # consume it, so m-tile 0 RIDES the norm pipeline (the PE
                # FIFO would otherwise stall behind all 8 ref chunks).
                # cur normalization runs one chunk AHEAD of the GEMM group
                # that consumes it (chunk ch feeds m-tiles 4ch..4ch+3 as
                # stationary), so its chain hides under 4 m-tiles of matmuls
                # instead of stalling PE at each chunk boundary.
                with tc.tile_pool(name="outp", bufs=3) as outp:
                    ndma = 0
                    for m in range(MT):
                        if m % (MT // NCH) == 0:
                            ch = m // (MT // NCH)
                            if ch + 1 < NCH:
                                norm_chunk("cur", ch + 1, steady=True)
                        msl = slice(m * P, (m + 1) * P)
                        ob = outp.tile([P, OBW], f16, tag="ob", name="ob")
                        # 2-bank psum tiles: 4 matmuls in, one wide copy out
                        for q in range(OBW // (2 * FD)):
                            pt = psp.tile(
                                [P, 2 * FD], f32, tag="pt", name="pt", bufs=4
                            )
                            for sub in range(2):
                                n = q * 2 + sub
                                nsl = slice(n * FD, (n + 1) * FD)
                                psl = slice(sub * FD, (sub + 1) * FD)
                                nc.tensor.matmul(
                                    pt[:, psl], scl["cur", 0][:, msl],
                                    scl["ref", 0][:, nsl],
                                    start=True, stop=False,
                                )
                                nc.tensor.matmul(
                                    pt[:, psl], scl["cur", 1][:, msl],
                                    scl["ref", 1][:, nsl],
                                    start=False, stop=True,
                                )
                            osl = slice(q * 2 * FD, (q + 1) * 2 * FD)
                            # evacuate (pure fp32->fp16 copy), balanced
                            # between ACT and DVE
                            if q % 2 == 0:
                                nc.scalar.activation(ob[:, osl], pt[:], AF.Copy)
                            else:
                                nc.vector.tensor_copy(ob[:, osl], pt[:])
                            # quarter-width output DMAs: earlier start,
                            # shorter tail; alternate the HWDGE rings
                            eng = nc.sync if ndma % 2 == 0 else nc.scalar
                            ndma += 1
                            eng.dma_start(out_d[msl, osl], ob[:, osl])

    nc.compile()
    return nc


def _get_nc():
    global _cached_nc
    if _cached_nc is None:
        _cached_nc = _build()
    return _cached_nc


def _run(cur, ref, trace=False, **kw):
    """cur/ref: [B, C, HW] float16. Returns (out [B, HW, HW] f16, results)."""
    nc = _get_nc()
    in_maps = [{"cur": cur[b], "ref": ref[b]} for b in range(B)]
    res = run_bass_kernel_spmd(nc, in_maps, list(range(B)), trace=trace, **kw)
    out = np.stack([np.asarray(res.results[b]["out"]) for b in range(B)])
    return out, res


def kernel(ref_features, cur_features):
    ref = np.ascontiguousarray(
        np.asarray(ref_features, np.float32).reshape(B, C, HW).astype(np.float16)
    )
    cur = np.ascontiguousarray(
        np.asarray(cur_features, np.float32).reshape(B, C, HW).astype(np.float16)
    )
    out, _ = _run(cur, ref)
    return out.astype(np.float32).reshape(B, H, W, H, W)
